# revision 22
# baseline (speedup 1.0000x reference)
"""MDTA (Restormer channel-attention block) on 8 TRN2 NeuronCores.

Sharding: (batch=2) x (4 row-blocks of 48 image rows) -> 8 cores.
Per core: 1x1 conv (bf16 matmul) -> depthwise 3x3 conv (per-channel
diagonal matmuls on the PE, zero-padded slab) -> Gram matrices
[q;k]^T[q;k] per head (contract over local pixels via PE transposes) ->
tiny AllReduce of Grams over the 4 cores of each batch -> softmax'd
channel attention -> attn @ v -> 1x1 proj. Only the 48x48 Grams cross
cores; everything else is local. Output is the core's (192, 48*192)
slice, concatenated on the host.
"""
import numpy as np
import ml_dtypes
from contextlib import ExitStack

import concourse.bass as bass
import concourse.tile as tile
import concourse.bacc as bacc
import concourse.mybir as mybir
from concourse import bass_utils

BF16 = mybir.dt.bfloat16
F32 = mybir.dt.float32
bf = ml_dtypes.bfloat16
AF = mybir.ActivationFunctionType

N_CORES = 8
C = 192                 # model dim
HEADS, HD = 4, 48
IMG = 192               # image h and w
RB = 48                 # image rows per core
PIX = RB * IMG          # 9216 valid pixels per core
SLABW = IMG + 2         # 194: padded row width
SLABR = RB + 2          # 50 slab rows (1 halo row each side)
SLABPIX = SLABR * SLABW  # 9700
NT = PIX // 384         # 24 row-pair tiles for conv/dwconv (384 px)
NTC = (SLABR * IMG) // 384   # 25 conv tiles over 9600 input pixels
NT2 = PIX // 512        # 18 tiles for attn@v / proj
EPS_NORM = 1e-12
EPS_TEMP = 1e-06

# channel chunks: q(0:192) k(192:384) v(384:576); v split 96/96 so the
# head boundaries stay partition-aligned for attn@v and proj.
CHUNKS = [(0, 128), (128, 128), (256, 128), (384, 96), (480, 96)]
TAPS = [(dy, dx) for dy in (-1, 0, 1) for dx in (-1, 0, 1)]

_cache = {}


def build_nc(reps: int = 1, single: bool = False, v_mode: str = 'dve'):
    nc = bacc.Bacc("TRN2", target_bir_lowering=False, debug=False,
                   num_devices=1 if single else N_CORES)
    x_t = nc.dram_tensor("x", [C, SLABR * IMG], BF16, kind="ExternalInput")
    wq_t = nc.dram_tensor("wqkvT", [C, 3 * C], BF16, kind="ExternalInput")
    wp_t = nc.dram_tensor("wprojT", [C, C], BF16, kind="ExternalInput")
    wd_t = nc.dram_tensor("wd", [128, 45], F32, kind="ExternalInput")
    id_t = nc.dram_tensor("ident", [128, 128], BF16, kind="ExternalInput")
    ey_t = nc.dram_tensor("eyet", [96, 4 * 96], F32, kind="ExternalInput")
    on_t = nc.dram_tensor("sel4", [HEADS, HEADS * HD], F32, kind="ExternalInput")
    tb_t = nc.dram_tensor("tempb", [96, HEADS], F32, kind="ExternalInput")
    out_t = nc.dram_tensor("out", [C, PIX], F32, kind="ExternalOutput")

    with tile.TileContext(nc) as tc:
        _body(tc, x_t, wq_t, wp_t, wd_t, id_t, ey_t, on_t, tb_t, out_t, reps,
              single, v_mode)
    nc.compile()
    return nc


def _body(tc, x_t, wq_t, wp_t, wd_t, id_t, ey_t, on_t, tb_t, out_t, reps,
          single=False, v_mode='dve'):
    nc = tc.nc
    with ExitStack() as ctx:
        P = ctx.enter_context(tc.tile_pool(name="persist", bufs=1))
        dram = ctx.enter_context(tc.tile_pool(name="dram", bufs=2,
                                              space="DRAM"))
        for _ in range(reps):
            _one_rep(tc, ctx, P, dram,
                     x_t, wq_t, wp_t, wd_t, id_t, ey_t, on_t, tb_t, out_t,
                     single, v_mode)


def _one_rep(tc, octx, P, dram,
             x_t, wq_t, wp_t, wd_t, id_t, ey_t, on_t, tb_t, out_t,
             single=False, v_mode='dve'):
    nc = tc.nc
    cp = [0]

    def copy(dst, src):
        # alternate PSUM->SBUF copies between scalar (ACT) and vector (DVE)
        eng = nc.scalar if cp[0] % 2 else nc.vector
        cp[0] += 1
        if eng is nc.scalar:
            nc.scalar.copy(dst, src)
        else:
            nc.vector.tensor_copy(dst, src)

    # ---- persistent tiles --------------------------------------------
    slab = [P.tile([mw, SLABPIX], BF16, tag=f"slab{i}", name=f"slab{i}")
            for i, (c0, mw) in enumerate(CHUNKS)]
    va = P.tile([96, PIX], BF16, tag="va")      # v ch 0..95   (heads 0,1)
    vb = P.tile([96, PIX], BF16, tag="vb")      # v ch 96..191 (heads 2,3)
    ndiag = 5 if v_mode == 'pe' else 3
    diag = [P.tile([mw, 9 * 128], BF16, tag=f"diag{i}", name=f"diag{i}")
            for i, (c0, mw) in enumerate(CHUNKS[:ndiag])]
    wq0 = P.tile([128, 3 * C], BF16, tag="wq0")
    wq1 = P.tile([64, 3 * C], BF16, tag="wq1")
    wpa = P.tile([96, C], BF16, tag="wpa")
    wpb = P.tile([96, C], BF16, tag="wpb")
    wd = P.tile([128, 45], F32, tag="wd")
    ident = P.tile([128, 128], BF16, tag="ident")
    eyet = P.tile([96, 4 * 96], F32, tag="eyet")
    sel4 = P.tile([HEADS, HEADS * HD], F32, tag="sel4")
    tempb = P.tile([96, HEADS], F32, tag="tempb")
    gsb = P.tile([96, 4 * 96], F32, tag="gsb")
    G = P.tile([96, 4 * 96], F32, tag="G")
    bd01 = P.tile([96, 96], BF16, tag="bd01")
    bd23 = P.tile([96, 96], BF16, tag="bd23")

    # critical-path weights first on the sync queue; late-needed consts
    # go on the gpsimd software-DGE queue so they don't delay x
    nc.sync.dma_start(wq0[:], wq_t.ap()[0:128, :])
    nc.sync.dma_start(wq1[:], wq_t.ap()[128:192, :])
    nc.gpsimd.dma_start(wd[:], wd_t.ap())
    nc.gpsimd.dma_start(ident[:], id_t.ap())
    nc.gpsimd.dma_start(wpa[:], wp_t.ap()[0:96, :])
    nc.gpsimd.dma_start(wpb[:], wp_t.ap()[96:192, :])
    nc.gpsimd.dma_start(eyet[:], ey_t.ap())
    nc.gpsimd.dma_start(sel4[:], on_t.ap())
    nc.gpsimd.dma_start(tempb[:], tb_t.ap())

    # diag weight tiles + slab pad zeroing
    for i, (c0, mw) in enumerate(CHUNKS):
        if i < ndiag:
            for t in range(9):
                nc.vector.tensor_scalar_mul(
                    diag[i][:, t * 128:(t + 1) * 128], ident[0:mw, :],
                    wd[0:mw, i * 9 + t: i * 9 + t + 1])
        v3 = slab[i][:, :].rearrange("p (r c) -> p r c", c=SLABW)
        nc.vector.memset(v3[:, :, 0:1], 0)
        nc.vector.memset(v3[:, :, SLABW - 1:SLABW], 0)

    with ExitStack() as ctx:
        xpool = ctx.enter_context(tc.tile_pool(name="xp", bufs=1))
        pp = ctx.enter_context(tc.tile_pool(name="pp", bufs=4, space="PSUM"))
        tp = ctx.enter_context(tc.tile_pool(name="tp", bufs=2, space="PSUM"))
        gp = ctx.enter_context(tc.tile_pool(name="gp", bufs=1, space="PSUM"))
        dp = ctx.enter_context(tc.tile_pool(name="dp", bufs=2))
        qp = ctx.enter_context(tc.tile_pool(name="qp", bufs=3))

        x0 = xpool.tile([128, SLABR * IMG], BF16, tag="x0")
        x1 = xpool.tile([64, SLABR * IMG], BF16, tag="x1")
        for j in range(10):
            js = slice(j * 960, (j + 1) * 960)
            nc.sync.dma_start(x0[:, js], x_t.ap()[0:128, js])
            nc.sync.dma_start(x1[:, js], x_t.ap()[128:192, js])

        # ---- 1x1 conv: slab[c] rows = W_qkv[c0:c0+mw] @ x -------------
        def conv_chunk(i, act_only=False):
            c0, mw = CHUNKS[i]
            v3 = slab[i][:, :].rearrange("p (r c) -> p r c", c=SLABW)
            for n in range(NTC):
                ps = pp.tile([128, 384], F32, tag="ps")
                sl = slice(n * 384, (n + 1) * 384)
                nc.tensor.matmul(ps[0:mw, :], wq0[:, c0:c0 + mw], x0[:, sl],
                                 start=True, stop=False)
                nc.tensor.matmul(ps[0:mw, :], wq1[:, c0:c0 + mw], x1[:, sl],
                                 start=False, stop=True)
                dst = v3[0:mw, 2 * n:2 * n + 2, 1:1 + IMG]
                if act_only:
                    nc.scalar.copy(dst, ps[0:mw, :])
                else:
                    copy(dst, ps[0:mw, :])

        # ---- depthwise conv helpers -----------------------------------
        def dw_rows(i, n):
            # rows 2n, 2n+1 of chunk i -> psum [mw, 2*192]
            mw = CHUNKS[i][1]
            v3 = slab[i][:, :].rearrange("p (r c) -> p r c", c=SLABW)
            ps = pp.tile([128, 384], F32, tag="ps")
            for r in (0, 1):
                y = 2 * n + r
                for t, (dy, dx) in enumerate(TAPS):
                    nc.tensor.matmul(
                        ps[:, r * IMG:(r + 1) * IMG],
                        diag[i][:, t * 128:(t + 1) * 128],
                        v3[:, y + 1 + dy, 1 + dx:1 + dx + IMG],
                        start=(t == 0), stop=(t == 8))
            return ps

        def v_chain(mult_eng):
            # v depthwise conv off the PE: per-channel scaled (tensor_scalar)
            # quarter-slabs + DVE accumulate adds
            vtp = ctx.enter_context(tc.tile_pool(name="vtp", bufs=2))
            QR = RB // 4
            for i, dst in ((3, va), (4, vb)):
                v3 = slab[i][:, :].rearrange("p (r c) -> p r c", c=SLABW)
                d3 = dst[:, :].rearrange("p (r c) -> p r c", c=IMG)
                for t, (dy, dx) in enumerate(TAPS):
                    wcol = wd[0:96, i * 9 + t:i * 9 + t + 1]
                    for qq in range(4):
                        r0 = qq * QR
                        src_ap = v3[:, 1 + dy + r0:1 + dy + r0 + QR,
                                    1 + dx:1 + dx + IMG]
                        if t == 0:
                            mult_eng.tensor_scalar_mul(
                                d3[:, r0:r0 + QR, :], src_ap, wcol)
                        else:
                            vt = vtp.tile([96, QR * IMG], BF16, tag="vt")
                            t3 = vt[:, :].rearrange("p (r c) -> p r c", c=IMG)
                            mult_eng.tensor_scalar_mul(t3[:, :, :], src_ap,
                                                       wcol)
                            hs = slice(r0 * IMG, (r0 + QR) * IMG)
                            nc.vector.tensor_add(dst[:, hs], dst[:, hs],
                                                 vt[:, :])

        # v first: its conv + off-PE depthwise overlap the q,k PE work
        if v_mode != 'pe':
            conv_chunk(3)
            conv_chunk(4)
            v_chain(nc.gpsimd if v_mode == 'pool' else nc.vector)
        for i in (0, 1, 2):
            conv_chunk(i, act_only=(v_mode != 'pe'))

        # ---- depthwise conv for q,k + transposes + Gram ---------------
        act_only = v_mode != 'pe'
        gps = gp.tile([128, 4 * 96], F32, tag="gram")
        for n in range(NT):
            dts = []
            for i in (0, 1, 2):
                ps = dw_rows(i, n)
                dt = dp.tile([128, 384], BF16, tag=f"d{i}", name=f"d{i}")
                if act_only:
                    nc.scalar.copy(dt[:], ps[:])
                else:
                    copy(dt[:], ps[:])
                dts.append(dt)
            for pb in range(3):
                tps = tp.tile([128, 384], BF16, tag="tps")
                for i in (0, 1, 2):
                    nc.tensor.transpose(
                        tps[:, i * 128:(i + 1) * 128],
                        dts[i][:, pb * 128:(pb + 1) * 128],
                        ident[:])
                qkt = qp.tile([128, 512], BF16, tag="qkt")
                # permute cols (t,h,d) -> (h,t,d) so each head's 96
                # columns [q_h | k_h] are contiguous for the Gram matmul
                qdst = qkt[:, 0:384].rearrange("p (h t d) -> p t h d",
                                               t=2, h=HEADS, d=HD)
                if act_only:
                    nc.scalar.copy(qdst, tps[:])
                    nc.scalar.memzero(qkt[:, 384:512])
                else:
                    copy(qdst, tps[:])
                    nc.vector.memset(qkt[:, 384:512], 0)
                first = (n == 0 and pb == 0)
                last = (n == NT - 1 and pb == 2)
                for h in range(HEADS):
                    # lhsT padded to 128 cols so FWL kicks in; junk output
                    # rows 96..127 never read
                    nc.tensor.matmul(gps[:, h * 96:(h + 1) * 96],
                                     qkt[:, h * 96:h * 96 + 128],
                                     qkt[:, h * 96:(h + 1) * 96],
                                     start=first, stop=last)
        copy(gsb[:], gps[0:96, :])

        # ---- AllReduce of Grams within each batch's 4 cores ----------
        if single:
            nc.vector.tensor_copy(G[:], gsb[:])
        else:
            arin = dram.tile([96, 4 * 96], F32, tag="arin")
            arout = dram.tile([96, 4 * 96], F32, tag="arout")
            nc.sync.dma_start(arin[:], gsb[:])
            nc.gpsimd.collective_compute(
                "AllReduce", mybir.AluOpType.add,
                replica_groups=[[0, 1, 2, 3], [4, 5, 6, 7]],
                ins=[arin.opt()], outs=[arout.opt()])
            nc.sync.dma_start(G[:], arout[:])

        # ---- norms, logits, softmax, attn^T --------------------------
        sp = ctx.enter_context(tc.tile_pool(name="sp", bufs=1))
        p2 = ctx.enter_context(tc.tile_pool(name="p2", bufs=1, space="PSUM"))

        gm = sp.tile([96, 4 * 96], F32, tag="gm")
        nc.vector.tensor_mul(gm[:], G[:], eyet[:])
        s_all = sp.tile([96, HEADS], F32, tag="s_all")
        for h in range(HEADS):
            nc.vector.tensor_reduce(s_all[:, h:h + 1],
                                    gm[:, h * 96:(h + 1) * 96],
                                    axis=mybir.AxisListType.X,
                                    op=mybir.AluOpType.add)
        nrm = sp.tile([96, HEADS], F32, tag="nrm")
        nc.scalar.sqrt(nrm[:], s_all[:])
        nc.vector.tensor_scalar_max(nrm[:], nrm[:], EPS_NORM)
        r_all = sp.tile([96, HEADS], F32, tag="r_all")
        nc.vector.reciprocal(r_all[:], nrm[:])
        nc.vector.tensor_mul(r_all[:], r_all[:], tempb[:])

        rtp = p2.tile([HEADS, 96], F32, tag="p2s")
        nc.tensor.transpose(rtp[:], r_all[:], eyet[:, 0:96])
        rT = sp.tile([HEADS, 96], F32, tag="rT")
        nc.vector.tensor_copy(rT[:], rtp[:])
        # rkb[d, 48h+e] = rk_h[e]: indicator lhsT selects head row of rT
        rkbp = p2.tile([HD, HEADS * HD], F32, tag="p2s")
        for h in range(HEADS):
            nc.tensor.matmul(rkbp[:, h * HD:(h + 1) * HD],
                             sel4[:, h * HD:(h + 1) * HD], rT[:, HD:96],
                             start=True, stop=True)
        rkb = sp.tile([HD, HEADS * HD], F32, tag="rkb")
        nc.vector.tensor_copy(rkb[:], rkbp[:])

        L = sp.tile([HD, HEADS * HD], F32, tag="L")
        for h in range(HEADS):
            nc.vector.tensor_scalar_mul(
                L[:, h * HD:(h + 1) * HD],
                G[0:HD, h * 96 + HD:(h + 1) * 96],
                r_all[0:HD, h:h + 1])
        nc.vector.tensor_mul(L[:], L[:], rkb[:])

        E = sp.tile([HD, HEADS * HD], F32, tag="E")
        den = sp.tile([HD, HEADS], F32, tag="den")
        for h in range(HEADS):
            nc.scalar.activation(E[:, h * HD:(h + 1) * HD],
                                 L[:, h * HD:(h + 1) * HD], AF.Exp,
                                 accum_out=den[:, h:h + 1])
        rd = sp.tile([HD, HEADS], F32, tag="rd")
        nc.vector.reciprocal(rd[:], den[:])
        A = sp.tile([HD, HEADS * HD], F32, tag="A")
        for h in range(HEADS):
            nc.vector.tensor_scalar_mul(A[:, h * HD:(h + 1) * HD],
                                        E[:, h * HD:(h + 1) * HD],
                                        rd[:, h:h + 1])
        # attn^T pairs -> blockdiag lhsT tiles for attn @ v. The second
        # 48x48 block starts at partition 48 (not 32-aligned), so it is
        # placed with an SBUF->SBUF DMA instead of an engine copy.
        for (bd, h0) in ((bd01, 0), (bd23, 2)):
            stp = p2.tile([96, HD], F32, tag="p2s")
            nc.tensor.transpose(stp[:], A[:, h0 * HD:(h0 + 2) * HD],
                                eyet[0:HD, 0:HD])
            stps = sp.tile([96, HD], BF16, tag="stps", name=f"stps{h0}")
            nc.vector.tensor_copy(stps[:], stp[:])
            nc.vector.memset(bd[:], 0)
            nc.vector.tensor_copy(bd[0:HD, 0:HD], stps[0:HD, :])
            nc.sync.dma_start(bd[HD:96, HD:96], stps[HD:96, :])

        # v on the PE (classic path): after the AR so the collective and
        # softmax chain hide under this compute
        if v_mode == 'pe':
            for i in (3, 4):
                conv_chunk(i)
            for n in range(NT):
                sl = slice(n * 384, (n + 1) * 384)
                for i, dst in ((3, va), (4, vb)):
                    mw = CHUNKS[i][1]
                    ps = dw_rows(i, n)
                    copy(dst[:, sl], ps[0:mw, :])

    # ---- attn @ v, proj, output --------------------------------------
    with ExitStack() as ctx:
        ap = ctx.enter_context(tc.tile_pool(name="ap", bufs=3))
        op = ctx.enter_context(tc.tile_pool(name="op", bufs=3))
        p3 = ctx.enter_context(tc.tile_pool(name="p3", bufs=2, space="PSUM"))
        for n in range(NT2):
            sl = slice(n * 512, (n + 1) * 512)
            pa = p3.tile([96, 512], F32, tag="pa")
            pb = p3.tile([96, 512], F32, tag="pb")
            nc.tensor.matmul(pa[:], bd01[:], va[:, sl], start=True, stop=True)
            nc.tensor.matmul(pb[:], bd23[:], vb[:, sl], start=True, stop=True)
            av_a = ap.tile([96, 512], BF16, tag="av_a")
            av_b = ap.tile([96, 512], BF16, tag="av_b")
            copy(av_a[:], pa[:])
            copy(av_b[:], pb[:])
            po0 = p3.tile([128, 512], F32, tag="po0")
            po1 = p3.tile([64, 512], F32, tag="po1")
            nc.tensor.matmul(po0[:], wpa[:, 0:128], av_a[:],
                             start=True, stop=False)
            nc.tensor.matmul(po0[:], wpb[:, 0:128], av_b[:],
                             start=False, stop=True)
            nc.tensor.matmul(po1[:], wpa[:, 128:192], av_a[:],
                             start=True, stop=False)
            nc.tensor.matmul(po1[:], wpb[:, 128:192], av_b[:],
                             start=False, stop=True)
            ot0 = op.tile([128, 512], F32, tag="ot0")
            ot1 = op.tile([64, 512], F32, tag="ot1")
            copy(ot0[:], po0[:])
            copy(ot1[:], po1[:])
            nc.sync.dma_start(out_t.ap()[0:128, sl], ot0[:])
            nc.sync.dma_start(out_t.ap()[128:192, sl], ot1[:])


# ---------------------------------------------------------------------
# host side
# ---------------------------------------------------------------------

def prep_inputs(x, w_qkv, w_dw, w_proj, log_temperature):
    """Build the 8 per-core input dicts."""
    x = np.asarray(x, np.float32)
    w_qkv = np.asarray(w_qkv, np.float32)
    w_dw = np.asarray(w_dw, np.float32).reshape(3 * C, 3, 3)
    w_proj = np.asarray(w_proj, np.float32)
    lt = np.asarray(log_temperature, np.float32).reshape(HEADS)

    wqkvT = np.ascontiguousarray(w_qkv.T).astype(bf)          # (192, 576)
    wprojT = np.ascontiguousarray(w_proj.T).astype(bf)        # (192, 192)
    wd = np.zeros((128, 45), np.float32)
    for i, (c0, mw) in enumerate(CHUNKS):
        for t, (dy, dx) in enumerate(TAPS):
            wd[0:mw, i * 9 + t] = w_dw[c0:c0 + mw, dy + 1, dx + 1]
    ident = np.eye(128).astype(bf)
    eyet = np.tile(np.eye(96, dtype=np.float32), (1, 4))
    eyet = np.ascontiguousarray(eyet)                          # (96, 384)
    sel4 = np.zeros((HEADS, HEADS * HD), np.float32)
    for h in range(HEADS):
        sel4[h, h * HD:(h + 1) * HD] = 1.0
    temp = np.log1p(np.exp(lt)) + EPS_TEMP
    tempb = np.ones((96, HEADS), np.float32)
    tempb[0:HD, :] = temp[None, :]

    in_maps = []
    for core in range(N_CORES):
        b, rb = core // 4, core % 4
        r0 = rb * RB
        slab = np.zeros((C, SLABR, IMG), np.float32)
        lo, hi = r0 - 1, r0 + RB + 1
        slo, shi = max(lo, 0), min(hi, IMG)
        slab[:, slo - lo:shi - lo, :] = x[b, :, slo:shi, :]
        in_maps.append({
            "x": np.ascontiguousarray(slab.reshape(C, SLABR * IMG)).astype(bf),
            "wqkvT": wqkvT, "wprojT": wprojT, "wd": wd, "ident": ident,
            "eyet": eyet, "sel4": sel4, "tempb": tempb,
        })
    return in_maps


def assemble(results):
    out = np.zeros((2, C, IMG, IMG), np.float32)
    for core in range(N_CORES):
        b, rb = core // 4, core % 4
        out[b, :, rb * RB:(rb + 1) * RB, :] = \
            results[core]["out"].reshape(C, RB, IMG)
    return out


def kernel(**inputs) -> np.ndarray:
    if "nc" not in _cache:
        _cache["nc"] = build_nc(reps=1)
    nc = _cache["nc"]
    in_maps = prep_inputs(**inputs)
    res = bass_utils.run_bass_kernel_spmd(
        nc, in_maps, core_ids=list(range(N_CORES)))
    return assemble(res.results)
